# revision 65
# baseline (speedup 1.0000x reference)
"""Multi-head attention (B=8, N=1024, C=768, H=12) for 8 Trainium2 NeuronCores.

Sharding: data-parallel over the batch dim — core b computes batch element b.
Weights are replicated; no collectives.

Everything runs bf16 (6.4e-3 max rel err vs the fp32 reference — matmuls
stream 1 col/cycle either way, so bf16 buys DMA bytes and SBUF, not stream
rate), and the PE is kept saturated end-to-end:
  - dummy warmup matmuls run while the first loads land, so the PE's DVFS
    ramp (1.2GHz for ~3us after idle) is spent on junk, not on the V GEMM;
  - the attention stream is ONE globally software-pipelined unit loop (96
    units, PV trailing by 4): seamless pair boundaries keep the ACT engine's
    EXP lead alive — the old per-pair loop left a 4-unit S hole per boundary
    that reset it. Pair j+1's qk GEMM interleaves into pair j's units on an
    explicit per-unit schedule (~2 matmuls/unit, O'-drain units left empty)
    — a unit with no extra PE work degrades to the EXP+semaphore cadence
    because the 2-deep S rotation couples the engines; pair 0's qk GEMM
    interleaves into the V GEMM's DMA-arrival gaps the same way;
  - the last pair, which has no qk GEMM to absorb, interleaves pair 4's
    normalize multiplies and proj tile nt=0's first 5 contraction chunks
    (ps_q is free there; phase E only finalizes the aT5 chunk);
  - V'' is padded to 128 rows/head so the PV accumulate writes the full
    partition width (measured ~20ns/matmul cheaper than 65);
  - mid-kernel weight prefetches ride the otherwise-idle gpsimd queue so a
    1.6us issue never delays the latency-serial normalize bounce chains on
    sync; everything latency-critical issues from hardware dge queues;
  - softmax denominators: DRAM-bounce spread -> [128,4] reciprocal ->
    bounce back, staged across TWO pair boundaries so no DVE op ever waits
    on an in-flight DMA (reciprocal_approx_fast would be 5x cheaper still,
    but produces NaN on real TRN2 hardware despite passing CoreSim);
  - GpSimd/Pool cannot touch PSUM on TRN2 (BIR verifier) — every PSUM drain
    lives on the DVE or ACT, scheduled so drains never queue behind casts.

Per-core plan (layouts picked so that NO on-device transposes are needed):
  host feeds xT=[C,N] bf16 (x[b].T), wqkp=[6*C,256] bf16 (per-pair [wq|wk]
  column blocks), wv=[C,C] bf16, wpT=[C,C] bf16, bproj=[C] f32.
  1. V GEMM:      V[n, dv]  = xT_chunk.T @ wv            (natural layout)
  2. qT/kT GEMM:  qkT[d, n] = wqk_chunk.T @ xT           (d on partitions)
  3. attention per head pair: S^T = kT_chunk.T @ qT ; expS = exp(SCALE*S^T) ;
     O'[66, nq] += V''_chunk.T @ expS  (row 64 = denom via the ones column)
  4. proj:        y[n, d2] = attnT_chunk.T @ wpT + bproj
"""

import sys

for _p in ("/opt/trn_rl_repo", "/opt/pypackages"):
    if _p not in sys.path:
        sys.path.append(_p)

import numpy as np

import concourse.bass as bass
import concourse.tile as tile
from concourse import bacc, mybir
from concourse.bass_utils import run_bass_kernel_spmd

B, N, C = 8, 1024, 768
H, HD = 12, 64
SCALE = HD**-0.5
NCORES = 8
KC = C // 128  # 6 contraction chunks over C
NT = N // 128  # 8 chunks over sequence (nk / n-tiles)
NQT = N // 512  # 2 moving-dim tiles over the query sequence
PAIRS = H // 2  # 6 head pairs
VW = 128  # V'' row width per head: 64 d + ones@64 + zero pad to full partitions
F32 = mybir.dt.float32
BF16 = mybir.dt.bfloat16
EXP = mybir.ActivationFunctionType.Exp
NWARM = 16  # dummy warmup matmuls (~3.5us) to hold the PE's DVFS at speed


def _emit(tc, nc, xT, wqkp, wv, wpT, bproj, y, ctx):
    persist = ctx.enter_context(tc.tile_pool(name="persist", bufs=1))
    wqk_pool = ctx.enter_context(tc.tile_pool(name="wqk", bufs=3))
    work = ctx.enter_context(tc.tile_pool(name="work", bufs=4))
    expp = ctx.enter_context(tc.tile_pool(name="expp", bufs=6))
    rdp = ctx.enter_context(tc.tile_pool(name="rdp", bufs=8))
    dram_scr = ctx.enter_context(tc.tile_pool(name="dram_scr", bufs=8, space="DRAM"))
    # 8 PSUM banks: ps_big = 2 x [128,1024] (2 banks each), ps_q = 2 x
    # [128,512] (1 bank each, qk psq halves), ps_o = 2 x [66,512] (1 bank
    # each, O' accumulators).
    ps_big = ctx.enter_context(tc.tile_pool(name="ps_big", bufs=2, space="PSUM"))
    ps_q = ctx.enter_context(tc.tile_pool(name="ps_q", bufs=2, space="PSUM"))
    ps_o = ctx.enter_context(tc.tile_pool(name="ps_o", bufs=2, space="PSUM"))

    # ---- persistent loads ----
    # One dma per [128,*] chunk tile, round-robin across the three DMA-capable
    # issue engines (a 128-row issue costs ~1.6us on the issuing engine, so
    # spreading issues is what bounds time-to-first-matmul). wv/xT ordered
    # kc-major so the V GEMM starts on chunk 0. After startup, gpsimd issues
    # NOTHING (its software dge queue's end-drain scales with issue count).
    eng3 = [nc.sync, nc.scalar, nc.gpsimd]
    # Warmup operand memset rides gpsimd as the very first instruction in
    # the program, so the PE's first warmup matmul is gated only by the
    # startup barrier, not by a queue of 1.6us DMA issues.
    dummy = persist.tile([128, 512], BF16, tag="warm")
    nc.gpsimd.memset(dummy, 0.0)
    nload = 0
    xTs = []
    wvs = []
    for kc in range(KC):
        tv = persist.tile([128, C], BF16, tag=f"wv{kc}", name=f"wv{kc}")
        tx = persist.tile([128, N], BF16, tag=f"xT{kc}", name=f"xT{kc}")
        eng3[nload % 3].dma_start(out=tv, in_=wv[kc * 128 : (kc + 1) * 128, :])
        nload += 1
        eng3[nload % 3].dma_start(out=tx, in_=xT[kc * 128 : (kc + 1) * 128, :])
        nload += 1
        xTs.append(tx)
        wvs.append(tv)
    bpb = persist.tile([128, C], F32, tag="bpb")
    nc.gpsimd.dma_start(
        out=bpb,
        in_=bass.AP(tensor=bproj.tensor, offset=bproj.offset, ap=[[0, 128]] + list(bproj.ap)),
    )

    def load_wqk(j, eng):
        # One dma for the whole pair: wqkp rows j*C:(j+1)*C are the [wq|wk]
        # [C,256] block; 3D AP folds the 6 contraction chunks into columns.
        t = wqk_pool.tile([128, KC * 256], BF16, tag="wqk", name=f"wqk{j}")
        eng.dma_start(
            out=t.rearrange("p (k c) -> p k c", c=256),
            in_=bass.AP(
                tensor=wqkp.tensor,
                offset=wqkp.offset + j * C * 256,
                ap=[[256, 128], [128 * 256, KC], [1, 256]],
            ),
        )
        return t

    wps = []

    def emit_wp_loads():
        # gpsimd: its queue is idle mid-attention, while a 1.6us issue on
        # sync would sit between the latency-serial normalize bounce hops.
        for kc in range(KC):
            t = persist.tile([128, C], BF16, tag=f"wp{kc}", name=f"wp{kc}")
            nc.gpsimd.dma_start(out=t, in_=wpT[kc * 128 : (kc + 1) * 128, :])
            wps.append(t)

    # ---- phase A: PE warmup on junk data while the loads land ----
    def emit_warmup():
        # Values are never read: the PSUM slots' next users open start=True.
        for i in range(NWARM):
            psd = ps_o.tile([128, 512], F32, tag="ps_o", name="warm")
            nc.tensor.matmul(psd, dummy[:, 0:128], dummy)

    # ---- phase B: V GEMM (natural layout, head-strided, ones + pad cols) ----
    v2s = []
    for nt in range(NT):
        v2s.append(persist.tile([128, H * VW], BF16, tag=f"v2{nt}", name=f"v2{nt}"))

    def emit_v(bg=(), bg_sched=()):
        # bg: pair-0 qk GEMM ops interleaved into the V stream (back-loaded —
        # their wqk tile lands ~13us in) to fill the DMA-arrival gaps the
        # PE would otherwise idle through.
        bgi = 0
        for nt in range(NT):
            psv = ps_big.tile([128, 1024], F32, tag="ps_big", name="psv")
            for kc in range(KC):
                lhsT = xTs[kc][:, nt * 128 : (nt + 1) * 128]
                nc.tensor.matmul(
                    psv[:, 0:512], lhsT, wvs[kc][:, 0:512], start=(kc == 0), stop=(kc == KC - 1)
                )
                nc.tensor.matmul(
                    psv[:, 512:768], lhsT, wvs[kc][:, 512:768], start=(kc == 0), stop=(kc == KC - 1)
                )
                if nt < 2:
                    # The first two nt sweeps are paced by chunk arrivals
                    # (~7.6us of DMA waits); junk matmuls between chunks keep
                    # the PE's DVFS streak alive so the real matmuls after
                    # each wait run at 2.4GHz instead of ramping from 1.2.
                    for _ in range(2):
                        psd = ps_o.tile([128, 512], F32, tag="ps_o", name="vfill")
                        nc.tensor.matmul(psd, dummy[:, 0:128], dummy)
            v2v = v2s[nt].rearrange("p (h e) -> p h e", e=VW)
            # ACT copy: the scalar engine is idle during phase B and drains
            # PSUM faster than the DVE (0.83 vs 1.04 ns/col).
            nc.scalar.copy(
                out=v2v[:, :, 0:HD], in_=psv[:, 0:768].rearrange("p (h e) -> p h e", e=HD)
            )
            nc.vector.memset(v2v[:, :, HD : HD + 1], 1.0)
            nc.vector.memset(v2v[:, :, HD + 1 : VW], 0.0)
            if bg:
                take = min(bg_sched[nt], len(bg) - bgi)
                for _ in range(take):
                    bg[bgi]()
                    bgi += 1
        while bgi < len(bg):
            bg[bgi]()
            bgi += 1

    # ---- phases C (qk GEMM) + D (attention) ----
    # qk GEMM for pair j+1 is emitted as a list of closures that emit_attn(j)
    # drains at ~2 matmuls per attention unit, keeping the PE saturated while
    # the ACT engine paces the EXP stream.
    def qk_ops(j, qT, kT, wt):
        # Op order exploits the ps_q 2-slot rotation: group g's psum slot is
        # only reallocated two groups later, so only the first two casts are
        # deadline-critical; casts 3+4 ride at the very end of the pair, far
        # from the O'-drain window, and are still done long before the next
        # pair's S matmuls read qT/kT.
        mms = []
        casts = []
        # k first, then q: attention's first S matmul needs q's cast last.
        for dst, base in ((kT, 128), (qT, 0)):
            for h2 in range(NQT):
                psq = []  # box for the psum tile, allocated by the first op

                def mk_mm(kc, dst=dst, base=base, h2=h2, psq=psq):
                    def op():
                        if kc == 0:
                            psq.append(ps_q.tile([128, 512], F32, tag="ps_q", name="psq"))
                        nc.tensor.matmul(
                            psq[0],
                            wt[:, kc * 256 + base : kc * 256 + base + 128],
                            xTs[kc][:, h2 * 512 : (h2 + 1) * 512],
                            start=(kc == 0),
                            stop=(kc == KC - 1),
                        )

                    return op

                def mk_cast(dst=dst, h2=h2, psq=psq):
                    def op():
                        nc.vector.tensor_copy(
                            out=dst[:, h2 * 512 : (h2 + 1) * 512], in_=psq[0]
                        )

                    return op

                mms.append([mk_mm(kc) for kc in range(KC)])
                casts.append(mk_cast())
        return (
            mms[0]
            + [casts[0]]
            + mms[1]
            + [casts[1]]
            + mms[2]
            + [casts[2]]
            + mms[3]
            + [casts[3]]
        )

    def emit_qk_block(j, qT, kT, wt):
        for op in qk_ops(j, qT, kT, wt):
            op()

    SKEW = 4

    def emit_norm1():
        for oc, rs, aT, half, nq, deng in pend1:
            rs2 = rdp.tile([128, 4], F32, tag="rs2")
            nc.vector.reciprocal(out=rs2, in_=rs)
            scr2 = dram_scr.tile([1, 512], F32, tag="scr2")
            deng.dma_start(
                out=bass.AP(tensor=scr2.tensor, offset=scr2.offset, ap=[[4, 128], [1, 4]]),
                in_=rs2,
            )
            rb = rdp.tile([64, 512], F32, tag="rb")
            deng.dma_start(
                out=rb,
                in_=bass.AP(
                    tensor=scr2.tensor, offset=scr2.offset, ap=[[0, 64]] + list(scr2.ap[1:])
                ),
            )
            pend2.append((oc, rb, aT, half, nq))
        pend1.clear()

    def emit_norm2():
        for oc, rb, aT, half, nq in pend2:
            nc.vector.tensor_mul(
                out=aT[half * 64 : half * 64 + 64, nq * 512 : (nq + 1) * 512],
                in0=oc[0:HD, :],
                in1=rb,
            )
        pend2.clear()

    proj_nt0 = {}

    def tail_ops():
        # pair 5 has no next-pair qk GEMM to interleave, so it runs at the
        # EXP-coupled cadence with the PE ~25% idle. Fill it: flush pair 4's
        # normalize multiplies (their bounces resolved by unit 2), then run
        # proj tile nt=0's first 5 contraction chunks on the free ps_q banks
        # — phase E only finalizes kc=5 once aT5 exists.
        ops = [emit_norm2]

        def mk(kc):
            def op():
                if kc == 0:
                    proj_nt0["a"] = ps_q.tile([128, 512], F32, tag="ps_q", name="psy512")
                    proj_nt0["b"] = ps_q.tile([128, 256], F32, tag="ps_q", name="psy256")
                lhsT = aTs[kc][:, 0:128]
                nc.tensor.matmul(
                    proj_nt0["a"],
                    lhsT,
                    wps[kc][:, 0:512],
                    start=(kc == 0),
                    stop=False,
                    skip_group_check=True,
                )
                nc.tensor.matmul(
                    proj_nt0["b"],
                    lhsT,
                    wps[kc][:, 512:768],
                    start=(kc == 0),
                    stop=False,
                    skip_group_check=True,
                )

            return op

        ops += [mk(kc) for kc in range(KC - 1)]
        # norm2 at r=4: pair 4's bounce-backs issue at r=0 (emit_norm1), so
        # flushing the multiplies any earlier stalls the DVE ~2us and the
        # proj partials (which read aT4's nq0 columns) stall behind it.
        sched = [0, 0, 0, 0, 1, 1, 1, 1, 1, 1, 0, 0, 0, 0, 0, 0]
        return ops, sched

    proj_nt1 = {}

    def mk_nt1(kc):
        # proj tile nt=1's aT0..4 chunks, run in the 4-unit PV-only trail at
        # stream end where the PE is otherwise nearly idle; its ps_big slot
        # comes free from the last S tile exactly then. Phase E finalizes
        # the aT5 chunk.
        def op():
            if kc == 0:
                proj_nt1["p"] = ps_big.tile([128, 1024], F32, tag="ps_big", name="psy")
            psy = proj_nt1["p"]
            lhsT = aTs[kc][:, 128:256]
            nc.tensor.matmul(
                psy[:, 0:512], lhsT, wps[kc][:, 0:512],
                start=(kc == 0), stop=False, skip_group_check=True,
            )
            nc.tensor.matmul(
                psy[:, 512:768], lhsT, wps[kc][:, 512:768],
                start=(kc == 0), stop=False, skip_group_check=True,
            )

        return op

    aTs = []
    qkts = []
    pend1 = []
    pend2 = []
    wts = {}
    for j in range(PAIRS):
        qkts.append(
            (
                persist.tile([128, N], BF16, tag=f"qT{j}", name=f"qT{j}"),
                persist.tile([128, N], BF16, tag=f"kT{j}", name=f"kT{j}"),
            )
        )
    for j in range(PAIRS):
        aTs.append(persist.tile([128, N], BF16, tag=f"aT{j}", name=f"aT{j}"))
    emit_warmup()
    wts[0] = load_wqk(0, nc.sync)
    wts[1] = load_wqk(1, nc.scalar)
    emit_v(qk_ops(0, *qkts[0], wts[0]), [0, 0, 2, 3, 4, 5, 6, 8])

    # ---- globally software-pipelined attention stream ----
    # One [128,1024] S tile per unit: head A in cols 0:512, head B in
    # 512:1024 — a single EXP covers both heads; PV trails SKEW units behind.
    # The unit stream is CONTINUOUS across head pairs: the old per-pair loop
    # left a 4-unit S hole at every boundary (the PV drain tail), which
    # reset the ACT engine's lead and made each pair's first units run at
    # the EXP+semaphore cadence. Background qk-GEMM schedule per pair: ~2
    # ops/unit (a unit with no bg work degrades to the EXP-coupled cadence),
    # O'-drain units 10-11 left empty so the DVE drain copies never queue
    # behind a cast, cast #3 pulled to unit 9, cast #4 landing by unit 14.
    QK_SCHED = [2, 2, 2, 2, 2, 2, 2, 3, 3, 3, 0, 0, 2, 2, 1, 1]
    trail_list = [mk_nt1(kc) for kc in range(KC - 1)]
    trail_i = 0
    UNITS = NQT * NT
    TOTAL = PAIRS * UNITS
    steps = {}
    oab = {}
    bgl, bgi, bgsched = [], 0, QK_SCHED
    for u in range(TOTAL + SKEW):
        if u < TOTAL:
            j, r = divmod(u, UNITS)
            if r == 0:
                if j + 2 < PAIRS:
                    wts[j + 2] = load_wqk(j + 2, nc.gpsimd)
                while bgi < len(bgl):  # leftovers from the previous pair
                    bgl[bgi]()
                    bgi += 1
                if j + 1 < PAIRS:
                    bgl, bgsched = qk_ops(j + 1, *qkts[j + 1], wts[j + 1]), QK_SCHED
                else:
                    bgl, bgsched = tail_ops()
                bgi = 0
                emit_norm2()  # pair j-2 multiplies (bounce-back long resolved)
                emit_norm1()  # pair j-1 reciprocal + bounce-back
                if j == 3:
                    # Late enough to stay off the qk-weight prefetch window,
                    # early enough (~60us before proj) to never gate it.
                    emit_wp_loads()
            nq, nkc = divmod(r, NT)
            qT, kT = qkts[j]
            s = ps_big.tile([128, 1024], F32, tag="ps_big", name="sAB")
            for half, kt0 in ((0, 0), (1, 64)):
                nc.tensor.matmul(
                    s[:, half * 512 : (half + 1) * 512],
                    kT[kt0 : kt0 + 64, nkc * 128 : (nkc + 1) * 128],
                    qT[kt0 : kt0 + 64, nq * 512 : (nq + 1) * 512],
                    tile_position=(kt0, 0),
                )
            e = expp.tile([128, 1024], BF16, tag="expp", name="eAB")
            nc.scalar.activation(out=e, in_=s, func=EXP, scale=SCALE)
            steps[u] = (j, nq, nkc, e)
        if u >= SKEW:
            j2, nq2, nkc2, e = steps.pop(u - SKEW)
            if nkc2 == 0:
                oab[(j2, nq2)] = (
                    ps_o.tile([VW, 512], F32, tag="ps_o", name="oA"),
                    ps_o.tile([VW, 512], F32, tag="ps_o", name="oB"),
                )
            oA, oB = oab.pop((j2, nq2)) if nkc2 == NT - 1 else oab[(j2, nq2)]
            v2v = v2s[nkc2].rearrange("p (h e) -> p h e", e=VW)
            nc.tensor.matmul(
                oA, v2v[:, 2 * j2, :], e[:, 0:512], start=(nkc2 == 0), stop=(nkc2 == NT - 1)
            )
            nc.tensor.matmul(
                oB,
                v2v[:, 2 * j2 + 1, :],
                e[:, 512:1024],
                start=(nkc2 == 0),
                stop=(nkc2 == NT - 1),
            )
            if nkc2 == NT - 1:
                for o, half in ((oA, 0), (oB, 1)):
                    # Drain O' to SBUF at once so the PSUM bank frees for the
                    # next nq tile. Remaining normalize work is staged across
                    # later pairs so no DVE op waits on a DMA. The very last
                    # drain's bounce rides the otherwise-idle gpsimd queue so
                    # the two chains don't serialize on sync's issue rate
                    # right before proj.
                    tail = j2 == PAIRS - 1 and nq2 == NQT - 1
                    deng = nc.gpsimd if (tail and half == 1) else nc.sync
                    oc = rdp.tile([HD + 1, 512], F32, tag="oc")
                    nc.vector.tensor_copy(out=oc, in_=o[0 : HD + 1, :])
                    scr = dram_scr.tile([1, 512], F32, tag="scr")
                    deng.dma_start(out=scr, in_=oc[HD : HD + 1, :])
                    rs = rdp.tile([128, 4], F32, tag="rs")
                    deng.dma_start(
                        out=rs,
                        in_=bass.AP(
                            tensor=scr.tensor, offset=scr.offset, ap=[[4, 128], [1, 4]]
                        ),
                    )
                    pend1.append((oc, rs, aTs[j2], half, nq2, deng))
        if u < TOTAL:
            take = min(bgsched[r], len(bgl) - bgi)
            for _ in range(take):
                bgl[bgi]()
                bgi += 1
            if u == TOTAL - 4:
                # Last pair: start nq=0's reciprocal + bounce-back now (the
                # DVE may briefly wait the spread — nothing else needs it
                # here, and the extra unit of slack keeps the bounce-back
                # ahead of the proj-head multiply even on a DMA-contended
                # core) ...
                emit_norm1()
            if u == TOTAL - 1:
                # ... and multiply 2 units later, once the bounce-back has
                # landed. aT5's first 512 columns are then ready the moment
                # the stream ends, so proj tiles nt=0..3 never stall on the
                # kc=5 contraction; nt=4..7 run ~12us later, by when nq=1's
                # chain has landed.
                emit_norm2()
        else:
            # PV-only trail units: fill the PE with proj nt=1's aT0..4
            # chunks (2 per unit; the 5th rides the first unit of phase E).
            for _ in range(2):
                if trail_i < len(trail_list):
                    trail_list[trail_i]()
                    trail_i += 1
    while bgi < len(bgl):
        bgl[bgi]()
        bgi += 1
    while trail_i < len(trail_list):
        trail_list[trail_i]()
        trail_i += 1
    # pair 5 nq=1's normalize flush is deferred INTO the proj loop (nt 2/3):
    # emitted here, its multiplies would stall the DVE on the just-issued
    # bounce DMAs exactly when proj's first bias-adds need it, backing up
    # the psy rotation. proj only reads those aT5 columns from nt=4 on.

    # ---- phase E: proj + bias ----
    # Attention is done, so ps_q and ps_o are free: alternating psy between
    # (ps_q+ps_o) and ps_big gives an effective rotation depth of 4.
    eng2 = [nc.sync, nc.scalar]
    # nt=1 first: its 10 aT0..4 matmuls run while pair 5 nq=0's normalize
    # bounce (a ~6us 4-hop chain from the last PV) lands, so nt=0's kc=5
    # finalize never stalls. nt=0 must still precede nt=2, which reuses its
    # ps_q slots.
    for nt in [1, 0] + list(range(2, NT)):
        if nt == 0:
            # kc 0..4 accumulated during pair 5's attention (tail_ops); only
            # the aT5 contraction chunk remains.
            ps512, ps256 = proj_nt0["a"], proj_nt0["b"]
            lhsT = aTs[KC - 1][:, 0:128]
            nc.tensor.matmul(
                ps512, lhsT, wps[KC - 1][:, 0:512], start=False, stop=True,
                skip_group_check=True,
            )
            nc.tensor.matmul(
                ps256, lhsT, wps[KC - 1][:, 512:768], start=False, stop=True,
                skip_group_check=True,
            )
        elif nt == 1:
            # kc 0..4 accumulated in the stream's trail units; finalize aT5.
            psy = proj_nt1["p"]
            ps512, ps256 = psy[:, 0:512], psy[:, 512:768]
            lhsT = aTs[KC - 1][:, 128:256]
            nc.tensor.matmul(
                ps512, lhsT, wps[KC - 1][:, 0:512], start=False, stop=True,
                skip_group_check=True,
            )
            nc.tensor.matmul(
                ps256, lhsT, wps[KC - 1][:, 512:768], start=False, stop=True,
                skip_group_check=True,
            )
        else:
            if nt % 2 == 0:
                ps512 = ps_q.tile([128, 512], F32, tag="ps_q", name="psy512")
                ps256 = ps_o.tile([128, 256], F32, tag="ps_o", name="psy256")
            else:
                psy = ps_big.tile([128, 1024], F32, tag="ps_big", name="psy")
                ps512, ps256 = psy[:, 0:512], psy[:, 512:768]
            for kc in range(KC):
                lhsT = aTs[kc][:, nt * 128 : (nt + 1) * 128]
                nc.tensor.matmul(
                    ps512, lhsT, wps[kc][:, 0:512], start=(kc == 0), stop=(kc == KC - 1)
                )
                nc.tensor.matmul(
                    ps256, lhsT, wps[kc][:, 512:768], start=(kc == 0), stop=(kc == KC - 1)
                )
        yb = work.tile([128, C], F32, tag="yb")
        nc.vector.tensor_add(out=yb[:, 0:512], in0=ps512, in1=bpb[:, 0:512])
        nc.vector.tensor_add(out=yb[:, 512:768], in0=ps256, in1=bpb[:, 512:768])
        for h in range(2):
            eng2[h].dma_start(
                out=y[nt * 128 : (nt + 1) * 128, h * 384 : (h + 1) * 384],
                in_=yb[:, h * 384 : (h + 1) * 384],
            )
        if nt == 1:
            emit_norm1()
        elif nt == 2:
            emit_norm2()


def build():
    from contextlib import ExitStack

    nc = bacc.Bacc("TRN2", target_bir_lowering=False, debug=False)
    xT = nc.dram_tensor("xT", [C, N], BF16, kind="ExternalInput").ap()
    wqkp = nc.dram_tensor("wqkp", [PAIRS * C, 256], BF16, kind="ExternalInput").ap()
    wv = nc.dram_tensor("wv", [C, C], BF16, kind="ExternalInput").ap()
    wpT = nc.dram_tensor("wpT", [C, C], BF16, kind="ExternalInput").ap()
    bproj = nc.dram_tensor("bproj", [C], F32, kind="ExternalInput").ap()
    y = nc.dram_tensor("y", [N, C], F32, kind="ExternalOutput").ap()
    with tile.TileContext(nc) as tc:
        with ExitStack() as ctx:
            _emit(tc, nc, xT, wqkp, wv, wpT, bproj, y, ctx)
    nc.compile()
    return nc


_NC_CACHE = {}


def make_in_maps(x, w_qkv, w_proj, b_proj):
    import ml_dtypes

    bf16 = ml_dtypes.bfloat16
    wqkvT = np.asarray(w_qkv).T  # [C, 3C]; cols 0:C=q, C:2C=k, 2C:3C=v
    blocks = [
        np.concatenate(
            [wqkvT[:, j * 128 : (j + 1) * 128], wqkvT[:, C + j * 128 : C + (j + 1) * 128]],
            axis=1,
        )
        for j in range(PAIRS)
    ]
    wqkp = np.ascontiguousarray(np.concatenate(blocks, axis=0)).astype(bf16)
    wv = np.ascontiguousarray(wqkvT[:, 2 * C : 3 * C]).astype(bf16)
    wpT = np.ascontiguousarray(np.asarray(w_proj).T).astype(bf16)
    b_proj = np.asarray(b_proj, dtype=np.float32)
    return [
        {
            "xT": np.ascontiguousarray(np.asarray(x[b]).T).astype(bf16),
            "wqkp": wqkp,
            "wv": wv,
            "wpT": wpT,
            "bproj": b_proj,
        }
        for b in range(NCORES)
    ]


def kernel(x, w_qkv, w_proj, b_proj, _trace=False, _tmpdir=None):
    if "nc" not in _NC_CACHE:
        _NC_CACHE["nc"] = build()
    nc = _NC_CACHE["nc"]
    in_maps = make_in_maps(x, w_qkv, w_proj, b_proj)
    kwargs = {}
    if _trace:
        kwargs = {"trace": True, "tmpdir": _tmpdir}
    res = run_bass_kernel_spmd(nc, in_maps, core_ids=list(range(NCORES)), **kwargs)
    out = np.stack([res.results[i]["y"] for i in range(NCORES)], axis=0)
    if _trace:
        _NC_CACHE["last_result"] = res
    return out


if __name__ == "__main__":
    rng = np.random.default_rng(0)
    x = rng.standard_normal((B, N, C), dtype=np.float32)
    w_qkv = (rng.standard_normal((3 * C, C), dtype=np.float32) * C**-0.5).astype(np.float32)
    w_proj = (rng.standard_normal((C, C), dtype=np.float32) * C**-0.5).astype(np.float32)
    b_proj = np.zeros(C, dtype=np.float32)
    out = kernel(x, w_qkv, w_proj, b_proj)
    print("out", out.shape, out.dtype, float(np.abs(out).mean()))


# revision 67
# speedup vs baseline: 1.2208x; 1.2208x over previous
"""Multi-head attention (B=8, N=1024, C=768, H=12) for 8 Trainium2 NeuronCores.

Sharding: data-parallel over the batch dim — core b computes batch element b.
Weights are replicated; no collectives.

Everything runs bf16 (6.4e-3 max rel err vs the fp32 reference — matmuls
stream 1 col/cycle either way, so bf16 buys DMA bytes and SBUF, not stream
rate), and the PE is kept saturated end-to-end:
  - dummy warmup matmuls run while the first loads land, so the PE's DVFS
    ramp (1.2GHz for ~3us after idle) is spent on junk, not on the V GEMM;
  - the attention stream is ONE globally software-pipelined unit loop (96
    units, PV trailing by 4): seamless pair boundaries keep the ACT engine's
    EXP lead alive — the old per-pair loop left a 4-unit S hole per boundary
    that reset it. Pair j+1's qk GEMM interleaves into pair j's units on an
    explicit per-unit schedule (~2 matmuls/unit, O'-drain units left empty)
    — a unit with no extra PE work degrades to the EXP+semaphore cadence
    because the 2-deep S rotation couples the engines; pair 0's qk GEMM
    interleaves into the V GEMM's DMA-arrival gaps the same way;
  - the last pair, which has no qk GEMM to absorb, interleaves pair 4's
    normalize multiplies and proj tile nt=0's first 5 contraction chunks
    (ps_q is free there; phase E only finalizes the aT5 chunk);
  - V'' is padded to 128 rows/head so the PV accumulate writes the full
    partition width (measured ~20ns/matmul cheaper than 65);
  - mid-kernel weight prefetches ride the otherwise-idle gpsimd queue so a
    1.6us issue never delays the latency-serial normalize bounce chains on
    sync; everything latency-critical issues from hardware dge queues;
  - softmax denominators: DRAM-bounce spread -> [128,4] reciprocal ->
    bounce back, staged across TWO pair boundaries so no DVE op ever waits
    on an in-flight DMA (reciprocal_approx_fast would be 5x cheaper still,
    but produces NaN on real TRN2 hardware despite passing CoreSim);
  - GpSimd/Pool cannot touch PSUM on TRN2 (BIR verifier) — every PSUM drain
    lives on the DVE or ACT, scheduled so drains never queue behind casts.

Per-core plan (layouts picked so that NO on-device transposes are needed):
  host feeds xT=[C,N] bf16 (x[b].T), wqkp=[6*C,256] bf16 (per-pair [wq|wk]
  column blocks), wv=[C,C] bf16, wpT=[C,C] bf16, bproj=[C] f32.
  1. V GEMM:      V[n, dv]  = xT_chunk.T @ wv            (natural layout)
  2. qT/kT GEMM:  qkT[d, n] = wqk_chunk.T @ xT           (d on partitions)
  3. attention per head pair: S^T = kT_chunk.T @ qT ; expS = exp(SCALE*S^T) ;
     O'[66, nq] += V''_chunk.T @ expS  (row 64 = denom via the ones column)
  4. proj:        y[n, d2] = attnT_chunk.T @ wpT + bproj
"""

import sys

for _p in ("/opt/trn_rl_repo", "/opt/pypackages"):
    if _p not in sys.path:
        sys.path.append(_p)

import numpy as np

import concourse.bass as bass
import concourse.tile as tile
from concourse import bacc, mybir
from concourse.bass_utils import run_bass_kernel_spmd

B, N, C = 8, 1024, 768
H, HD = 12, 64
SCALE = HD**-0.5
NCORES = 8
KC = C // 128  # 6 contraction chunks over C
NT = N // 128  # 8 chunks over sequence (nk / n-tiles)
NQT = N // 512  # 2 moving-dim tiles over the query sequence
PAIRS = H // 2  # 6 head pairs
VW = 128  # V'' row width per head: 64 d + ones@64 + zero pad to full partitions
F32 = mybir.dt.float32
BF16 = mybir.dt.bfloat16
EXP = mybir.ActivationFunctionType.Exp
NWARM = 16  # dummy warmup matmuls (~3.5us) to hold the PE's DVFS at speed


def _emit(tc, nc, xT, wqkp, wv, wpT, bproj, y, ctx):
    persist = ctx.enter_context(tc.tile_pool(name="persist", bufs=1))
    wqk_pool = ctx.enter_context(tc.tile_pool(name="wqk", bufs=3))
    work = ctx.enter_context(tc.tile_pool(name="work", bufs=4))
    expp = ctx.enter_context(tc.tile_pool(name="expp", bufs=6))
    rdp = ctx.enter_context(tc.tile_pool(name="rdp", bufs=8))
    dram_scr = ctx.enter_context(tc.tile_pool(name="dram_scr", bufs=8, space="DRAM"))
    # 8 PSUM banks: ps_big = 2 x [128,1024] (2 banks each), ps_q = 2 x
    # [128,512] (1 bank each, qk psq halves), ps_o = 2 x [66,512] (1 bank
    # each, O' accumulators).
    ps_big = ctx.enter_context(tc.tile_pool(name="ps_big", bufs=2, space="PSUM"))
    ps_q = ctx.enter_context(tc.tile_pool(name="ps_q", bufs=2, space="PSUM"))
    ps_o = ctx.enter_context(tc.tile_pool(name="ps_o", bufs=2, space="PSUM"))

    # ---- persistent loads ----
    # One dma per [128,*] chunk tile, round-robin across the three DMA-capable
    # issue engines (a 128-row issue costs ~1.6us on the issuing engine, so
    # spreading issues is what bounds time-to-first-matmul). wv/xT ordered
    # kc-major so the V GEMM starts on chunk 0. After startup, gpsimd issues
    # NOTHING (its software dge queue's end-drain scales with issue count).
    eng3 = [nc.sync, nc.scalar, nc.gpsimd]
    # Warmup operand memset rides gpsimd as the very first instruction in
    # the program, so the PE's first warmup matmul is gated only by the
    # startup barrier, not by a queue of 1.6us DMA issues.
    dummy = persist.tile([128, 512], BF16, tag="warm")
    nc.gpsimd.memset(dummy, 0.0)
    nload = 0
    xTs = []
    wvs = []
    for kc in range(KC):
        tv = persist.tile([128, C], BF16, tag=f"wv{kc}", name=f"wv{kc}")
        tx = persist.tile([128, N], BF16, tag=f"xT{kc}", name=f"xT{kc}")
        eng3[nload % 3].dma_start(out=tv, in_=wv[kc * 128 : (kc + 1) * 128, :])
        nload += 1
        eng3[nload % 3].dma_start(out=tx, in_=xT[kc * 128 : (kc + 1) * 128, :])
        nload += 1
        xTs.append(tx)
        wvs.append(tv)
    bpb = persist.tile([128, C], F32, tag="bpb")
    nc.gpsimd.dma_start(
        out=bpb,
        in_=bass.AP(tensor=bproj.tensor, offset=bproj.offset, ap=[[0, 128]] + list(bproj.ap)),
    )

    def load_wqk(j, eng):
        # One dma for the whole pair: wqkp rows j*C:(j+1)*C are the [wq|wk]
        # [C,256] block; 3D AP folds the 6 contraction chunks into columns.
        t = wqk_pool.tile([128, KC * 256], BF16, tag="wqk", name=f"wqk{j}")
        eng.dma_start(
            out=t.rearrange("p (k c) -> p k c", c=256),
            in_=bass.AP(
                tensor=wqkp.tensor,
                offset=wqkp.offset + j * C * 256,
                ap=[[256, 128], [128 * 256, KC], [1, 256]],
            ),
        )
        return t

    wps = []

    def emit_wp_loads():
        # gpsimd: its queue is idle mid-attention, while a 1.6us issue on
        # sync would sit between the latency-serial normalize bounce hops.
        for kc in range(KC):
            t = persist.tile([128, C], BF16, tag=f"wp{kc}", name=f"wp{kc}")
            nc.gpsimd.dma_start(out=t, in_=wpT[kc * 128 : (kc + 1) * 128, :])
            wps.append(t)

    # ---- phase A: PE warmup on junk data while the loads land ----
    def emit_warmup():
        # Values are never read: the PSUM slots' next users open start=True.
        for i in range(NWARM):
            psd = ps_o.tile([128, 512], F32, tag="ps_o", name="warm")
            nc.tensor.matmul(psd, dummy[:, 0:128], dummy)

    # ---- phase B: V GEMM (natural layout, head-strided, ones + pad cols) ----
    v2s = []
    for nt in range(NT):
        v2s.append(persist.tile([128, H * VW], BF16, tag=f"v2{nt}", name=f"v2{nt}"))

    def emit_v(bg=(), bg_sched=()):
        # bg: pair-0 qk GEMM ops interleaved into the V stream (back-loaded —
        # their wqk tile lands ~13us in) to fill the DMA-arrival gaps the
        # PE would otherwise idle through.
        bgi = 0
        for nt in range(NT):
            psv = ps_big.tile([128, 1024], F32, tag="ps_big", name="psv")
            for kc in range(KC):
                lhsT = xTs[kc][:, nt * 128 : (nt + 1) * 128]
                nc.tensor.matmul(
                    psv[:, 0:512], lhsT, wvs[kc][:, 0:512], start=(kc == 0), stop=(kc == KC - 1)
                )
                nc.tensor.matmul(
                    psv[:, 512:768], lhsT, wvs[kc][:, 512:768], start=(kc == 0), stop=(kc == KC - 1)
                )
                if nt < 2:
                    # The first two nt sweeps are paced by chunk arrivals
                    # (~7.6us of DMA waits); junk matmuls between chunks keep
                    # the PE's DVFS streak alive so the real matmuls after
                    # each wait run at 2.4GHz instead of ramping from 1.2.
                    for _ in range(2):
                        psd = ps_o.tile([128, 512], F32, tag="ps_o", name="vfill")
                        nc.tensor.matmul(psd, dummy[:, 0:128], dummy)
            v2v = v2s[nt].rearrange("p (h e) -> p h e", e=VW)
            # ACT copy: the scalar engine is idle during phase B and drains
            # PSUM faster than the DVE (0.83 vs 1.04 ns/col).
            nc.scalar.copy(
                out=v2v[:, :, 0:HD], in_=psv[:, 0:768].rearrange("p (h e) -> p h e", e=HD)
            )
            nc.vector.memset(v2v[:, :, HD : HD + 1], 1.0)
            nc.vector.memset(v2v[:, :, HD + 1 : VW], 0.0)
            if bg:
                take = min(bg_sched[nt], len(bg) - bgi)
                for _ in range(take):
                    bg[bgi]()
                    bgi += 1
        while bgi < len(bg):
            bg[bgi]()
            bgi += 1

    # ---- phases C (qk GEMM) + D (attention) ----
    # qk GEMM for pair j+1 is emitted as a list of closures that emit_attn(j)
    # drains at ~2 matmuls per attention unit, keeping the PE saturated while
    # the ACT engine paces the EXP stream.
    def qk_ops(j, qT, kT, wt):
        # Op order exploits the ps_q 2-slot rotation: group g's psum slot is
        # only reallocated two groups later, so only the first two casts are
        # deadline-critical; casts 3+4 ride at the very end of the pair, far
        # from the O'-drain window, and are still done long before the next
        # pair's S matmuls read qT/kT.
        mms = []
        casts = []
        # k first, then q: attention's first S matmul needs q's cast last.
        for dst, base in ((kT, 128), (qT, 0)):
            for h2 in range(NQT):
                psq = []  # box for the psum tile, allocated by the first op

                def mk_mm(kc, dst=dst, base=base, h2=h2, psq=psq):
                    def op():
                        if kc == 0:
                            psq.append(ps_q.tile([128, 512], F32, tag="ps_q", name="psq"))
                        nc.tensor.matmul(
                            psq[0],
                            wt[:, kc * 256 + base : kc * 256 + base + 128],
                            xTs[kc][:, h2 * 512 : (h2 + 1) * 512],
                            start=(kc == 0),
                            stop=(kc == KC - 1),
                        )

                    return op

                def mk_cast(dst=dst, h2=h2, psq=psq):
                    def op():
                        nc.vector.tensor_copy(
                            out=dst[:, h2 * 512 : (h2 + 1) * 512], in_=psq[0]
                        )

                    return op

                mms.append([mk_mm(kc) for kc in range(KC)])
                casts.append(mk_cast())
        return (
            mms[0]
            + [casts[0]]
            + mms[1]
            + [casts[1]]
            + mms[2]
            + [casts[2]]
            + mms[3]
            + [casts[3]]
        )

    def emit_qk_block(j, qT, kT, wt):
        for op in qk_ops(j, qT, kT, wt):
            op()

    SKEW = 4

    def emit_norm1():
        for oc, rs, aT, half, nq, deng in pend1:
            rs2 = rdp.tile([128, 4], F32, tag="rs2")
            nc.vector.reciprocal(out=rs2, in_=rs)
            scr2 = dram_scr.tile([1, 512], F32, tag="scr2")
            deng.dma_start(
                out=bass.AP(tensor=scr2.tensor, offset=scr2.offset, ap=[[4, 128], [1, 4]]),
                in_=rs2,
            )
            rb = rdp.tile([64, 512], F32, tag="rb")
            deng.dma_start(
                out=rb,
                in_=bass.AP(
                    tensor=scr2.tensor, offset=scr2.offset, ap=[[0, 64]] + list(scr2.ap[1:])
                ),
            )
            pend2.append((oc, rb, aT, half, nq))
        pend1.clear()

    def emit_norm2():
        for oc, rb, aT, half, nq in pend2:
            nc.vector.tensor_mul(
                out=aT[half * 64 : half * 64 + 64, nq * 512 : (nq + 1) * 512],
                in0=oc[0:HD, :],
                in1=rb,
            )
        pend2.clear()

    proj_nt0 = {}

    def tail_ops():
        # pair 5 has no next-pair qk GEMM to interleave, so it runs at the
        # EXP-coupled cadence with the PE ~25% idle. Fill it: flush pair 4's
        # normalize multiplies (their bounces resolved by unit 2), then run
        # proj tile nt=0's first 5 contraction chunks on the free ps_q banks
        # — phase E only finalizes kc=5 once aT5 exists.
        ops = [emit_norm2]

        def mk(kc):
            def op():
                if kc == 0:
                    proj_nt0["a"] = ps_q.tile([128, 512], F32, tag="ps_q", name="psy512")
                    proj_nt0["b"] = ps_q.tile([128, 256], F32, tag="ps_q", name="psy256")
                lhsT = aTs[kc][:, 0:128]
                nc.tensor.matmul(
                    proj_nt0["a"],
                    lhsT,
                    wps[kc][:, 0:512],
                    start=(kc == 0),
                    stop=False,
                    skip_group_check=True,
                )
                nc.tensor.matmul(
                    proj_nt0["b"],
                    lhsT,
                    wps[kc][:, 512:768],
                    start=(kc == 0),
                    stop=False,
                    skip_group_check=True,
                )

            return op

        ops += [mk(kc) for kc in range(KC - 1)]
        # norm2 at r=4: pair 4's bounce-backs issue at r=0 (emit_norm1), so
        # flushing the multiplies any earlier stalls the DVE ~2us and the
        # proj partials (which read aT4's nq0 columns) stall behind it.
        sched = [0, 0, 0, 0, 1, 1, 1, 1, 1, 1, 0, 0, 0, 0, 0, 0]
        return ops, sched

    aTs = []
    qkts = []
    pend1 = []
    pend2 = []
    wts = {}
    for j in range(PAIRS):
        qkts.append(
            (
                persist.tile([128, N], BF16, tag=f"qT{j}", name=f"qT{j}"),
                persist.tile([128, N], BF16, tag=f"kT{j}", name=f"kT{j}"),
            )
        )
    for j in range(PAIRS):
        aTs.append(persist.tile([128, N], BF16, tag=f"aT{j}", name=f"aT{j}"))
    emit_warmup()
    wts[0] = load_wqk(0, nc.sync)
    wts[1] = load_wqk(1, nc.scalar)
    emit_v(qk_ops(0, *qkts[0], wts[0]), [0, 0, 2, 3, 4, 5, 6, 8])

    # ---- globally software-pipelined attention stream ----
    # One [128,1024] S tile per unit: head A in cols 0:512, head B in
    # 512:1024 — a single EXP covers both heads; PV trails SKEW units behind.
    # The unit stream is CONTINUOUS across head pairs: the old per-pair loop
    # left a 4-unit S hole at every boundary (the PV drain tail), which
    # reset the ACT engine's lead and made each pair's first units run at
    # the EXP+semaphore cadence. Background qk-GEMM schedule per pair: ~2
    # ops/unit (a unit with no bg work degrades to the EXP-coupled cadence),
    # O'-drain units 10-11 left empty so the DVE drain copies never queue
    # behind a cast, cast #3 pulled to unit 9, cast #4 landing by unit 14.
    QK_SCHED = [2, 2, 2, 2, 2, 2, 2, 3, 3, 3, 0, 0, 2, 2, 1, 1]
    UNITS = NQT * NT
    TOTAL = PAIRS * UNITS
    steps = {}
    oab = {}
    bgl, bgi, bgsched = [], 0, QK_SCHED
    for u in range(TOTAL + SKEW):
        if u < TOTAL:
            j, r = divmod(u, UNITS)
            if r == 0:
                if j + 2 < PAIRS:
                    wts[j + 2] = load_wqk(j + 2, nc.gpsimd)
                while bgi < len(bgl):  # leftovers from the previous pair
                    bgl[bgi]()
                    bgi += 1
                if j + 1 < PAIRS:
                    bgl, bgsched = qk_ops(j + 1, *qkts[j + 1], wts[j + 1]), QK_SCHED
                else:
                    bgl, bgsched = tail_ops()
                bgi = 0
                emit_norm2()  # pair j-2 multiplies (bounce-back long resolved)
                emit_norm1()  # pair j-1 reciprocal + bounce-back
                if j == 3:
                    # Late enough to stay off the qk-weight prefetch window,
                    # early enough (~60us before proj) to never gate it.
                    emit_wp_loads()
            nq, nkc = divmod(r, NT)
            qT, kT = qkts[j]
            s = ps_big.tile([128, 1024], F32, tag="ps_big", name="sAB")
            for half, kt0 in ((0, 0), (1, 64)):
                nc.tensor.matmul(
                    s[:, half * 512 : (half + 1) * 512],
                    kT[kt0 : kt0 + 64, nkc * 128 : (nkc + 1) * 128],
                    qT[kt0 : kt0 + 64, nq * 512 : (nq + 1) * 512],
                    tile_position=(kt0, 0),
                )
            e = expp.tile([128, 1024], BF16, tag="expp", name="eAB")
            nc.scalar.activation(out=e, in_=s, func=EXP, scale=SCALE)
            steps[u] = (j, nq, nkc, e)
        if u >= SKEW:
            j2, nq2, nkc2, e = steps.pop(u - SKEW)
            if nkc2 == 0:
                oab[(j2, nq2)] = (
                    ps_o.tile([VW, 512], F32, tag="ps_o", name="oA"),
                    ps_o.tile([VW, 512], F32, tag="ps_o", name="oB"),
                )
            oA, oB = oab.pop((j2, nq2)) if nkc2 == NT - 1 else oab[(j2, nq2)]
            v2v = v2s[nkc2].rearrange("p (h e) -> p h e", e=VW)
            nc.tensor.matmul(
                oA, v2v[:, 2 * j2, :], e[:, 0:512], start=(nkc2 == 0), stop=(nkc2 == NT - 1)
            )
            nc.tensor.matmul(
                oB,
                v2v[:, 2 * j2 + 1, :],
                e[:, 512:1024],
                start=(nkc2 == 0),
                stop=(nkc2 == NT - 1),
            )
            if nkc2 == NT - 1:
                for o, half in ((oA, 0), (oB, 1)):
                    # Drain O' to SBUF at once so the PSUM bank frees for the
                    # next nq tile. Remaining normalize work is staged across
                    # later pairs so no DVE op waits on a DMA. The very last
                    # drain's bounce rides the otherwise-idle gpsimd queue so
                    # the two chains don't serialize on sync's issue rate
                    # right before proj.
                    tail = j2 == PAIRS - 1 and nq2 == NQT - 1
                    deng = nc.gpsimd if (tail and half == 1) else nc.sync
                    oc = rdp.tile([HD + 1, 512], F32, tag="oc")
                    nc.vector.tensor_copy(out=oc, in_=o[0 : HD + 1, :])
                    scr = dram_scr.tile([1, 512], F32, tag="scr")
                    deng.dma_start(out=scr, in_=oc[HD : HD + 1, :])
                    rs = rdp.tile([128, 4], F32, tag="rs")
                    deng.dma_start(
                        out=rs,
                        in_=bass.AP(
                            tensor=scr.tensor, offset=scr.offset, ap=[[4, 128], [1, 4]]
                        ),
                    )
                    pend1.append((oc, rs, aTs[j2], half, nq2, deng))
        if u < TOTAL:
            take = min(bgsched[r], len(bgl) - bgi)
            for _ in range(take):
                bgl[bgi]()
                bgi += 1
            if u == TOTAL - 4:
                # Last pair: start nq=0's reciprocal + bounce-back now (the
                # DVE may briefly wait the spread — nothing else needs it
                # here, and the extra unit of slack keeps the bounce-back
                # ahead of the proj-head multiply even on a DMA-contended
                # core) ...
                emit_norm1()
            if u == TOTAL - 1:
                # ... and multiply 2 units later, once the bounce-back has
                # landed. aT5's first 512 columns are then ready the moment
                # the stream ends, so proj tiles nt=0..3 never stall on the
                # kc=5 contraction; nt=4..7 run ~12us later, by when nq=1's
                # chain has landed.
                emit_norm2()
    while bgi < len(bgl):
        bgl[bgi]()
        bgi += 1
    # pair 5 nq=1's normalize flush is deferred INTO the proj loop (nt 2/3):
    # emitted here, its multiplies would stall the DVE on the just-issued
    # bounce DMAs exactly when proj's first bias-adds need it, backing up
    # the psy rotation. proj only reads those aT5 columns from nt=4 on.

    # ---- phase E: proj + bias ----
    # Attention is done, so ps_q and ps_o are free: alternating psy between
    # (ps_q+ps_o) and ps_big gives an effective rotation depth of 4.
    eng2 = [nc.sync, nc.scalar]
    # nt=1 first: its 10 aT0..4 matmuls run while pair 5 nq=0's normalize
    # bounce (a ~6us 4-hop chain from the last PV) lands, so nt=0's kc=5
    # finalize never stalls. nt=0 must still precede nt=2, which reuses its
    # ps_q slots.
    for nt in [1, 0] + list(range(2, NT)):
        if nt == 0:
            # kc 0..4 accumulated during pair 5's attention (tail_ops); only
            # the aT5 contraction chunk remains.
            ps512, ps256 = proj_nt0["a"], proj_nt0["b"]
            lhsT = aTs[KC - 1][:, 0:128]
            nc.tensor.matmul(
                ps512, lhsT, wps[KC - 1][:, 0:512], start=False, stop=True,
                skip_group_check=True,
            )
            nc.tensor.matmul(
                ps256, lhsT, wps[KC - 1][:, 512:768], start=False, stop=True,
                skip_group_check=True,
            )
        else:
            if nt % 2 == 0:
                ps512 = ps_q.tile([128, 512], F32, tag="ps_q", name="psy512")
                ps256 = ps_o.tile([128, 256], F32, tag="ps_o", name="psy256")
            else:
                psy = ps_big.tile([128, 1024], F32, tag="ps_big", name="psy")
                ps512, ps256 = psy[:, 0:512], psy[:, 512:768]
            for kc in range(KC):
                lhsT = aTs[kc][:, nt * 128 : (nt + 1) * 128]
                nc.tensor.matmul(
                    ps512, lhsT, wps[kc][:, 0:512], start=(kc == 0), stop=(kc == KC - 1)
                )
                nc.tensor.matmul(
                    ps256, lhsT, wps[kc][:, 512:768], start=(kc == 0), stop=(kc == KC - 1)
                )
        yb = work.tile([128, C], BF16, tag="yb")
        nc.vector.tensor_add(out=yb[:, 0:512], in0=ps512, in1=bpb[:, 0:512])
        nc.vector.tensor_add(out=yb[:, 512:768], in0=ps256, in1=bpb[:, 512:768])
        for h in range(2):
            eng2[h].dma_start(
                out=y[nt * 128 : (nt + 1) * 128, h * 384 : (h + 1) * 384],
                in_=yb[:, h * 384 : (h + 1) * 384],
            )
        if nt == 1:
            emit_norm1()
        elif nt == 2:
            emit_norm2()


def build():
    from contextlib import ExitStack

    nc = bacc.Bacc("TRN2", target_bir_lowering=False, debug=False)
    xT = nc.dram_tensor("xT", [C, N], BF16, kind="ExternalInput").ap()
    wqkp = nc.dram_tensor("wqkp", [PAIRS * C, 256], BF16, kind="ExternalInput").ap()
    wv = nc.dram_tensor("wv", [C, C], BF16, kind="ExternalInput").ap()
    wpT = nc.dram_tensor("wpT", [C, C], BF16, kind="ExternalInput").ap()
    bproj = nc.dram_tensor("bproj", [C], F32, kind="ExternalInput").ap()
    y = nc.dram_tensor("y", [N, C], BF16, kind="ExternalOutput").ap()
    with tile.TileContext(nc) as tc:
        with ExitStack() as ctx:
            _emit(tc, nc, xT, wqkp, wv, wpT, bproj, y, ctx)
    nc.compile()
    return nc


_NC_CACHE = {}


def make_in_maps(x, w_qkv, w_proj, b_proj):
    import ml_dtypes

    bf16 = ml_dtypes.bfloat16
    wqkvT = np.asarray(w_qkv).T  # [C, 3C]; cols 0:C=q, C:2C=k, 2C:3C=v
    blocks = [
        np.concatenate(
            [wqkvT[:, j * 128 : (j + 1) * 128], wqkvT[:, C + j * 128 : C + (j + 1) * 128]],
            axis=1,
        )
        for j in range(PAIRS)
    ]
    wqkp = np.ascontiguousarray(np.concatenate(blocks, axis=0)).astype(bf16)
    wv = np.ascontiguousarray(wqkvT[:, 2 * C : 3 * C]).astype(bf16)
    wpT = np.ascontiguousarray(np.asarray(w_proj).T).astype(bf16)
    b_proj = np.asarray(b_proj, dtype=np.float32)
    return [
        {
            "xT": np.ascontiguousarray(np.asarray(x[b]).T).astype(bf16),
            "wqkp": wqkp,
            "wv": wv,
            "wpT": wpT,
            "bproj": b_proj,
        }
        for b in range(NCORES)
    ]


def kernel(x, w_qkv, w_proj, b_proj, _trace=False, _tmpdir=None):
    if "nc" not in _NC_CACHE:
        _NC_CACHE["nc"] = build()
    nc = _NC_CACHE["nc"]
    in_maps = make_in_maps(x, w_qkv, w_proj, b_proj)
    kwargs = {}
    if _trace:
        kwargs = {"trace": True, "tmpdir": _tmpdir}
    res = run_bass_kernel_spmd(nc, in_maps, core_ids=list(range(NCORES)), **kwargs)
    out = np.stack([res.results[i]["y"] for i in range(NCORES)], axis=0).astype(np.float32)
    if _trace:
        _NC_CACHE["last_result"] = res
    return out


if __name__ == "__main__":
    rng = np.random.default_rng(0)
    x = rng.standard_normal((B, N, C), dtype=np.float32)
    w_qkv = (rng.standard_normal((3 * C, C), dtype=np.float32) * C**-0.5).astype(np.float32)
    w_proj = (rng.standard_normal((C, C), dtype=np.float32) * C**-0.5).astype(np.float32)
    b_proj = np.zeros(C, dtype=np.float32)
    out = kernel(x, w_qkv, w_proj, b_proj)
    print("out", out.shape, out.dtype, float(np.abs(out).mean()))
